# revision 21
# baseline (speedup 1.0000x reference)
"""CrossConsensus kernel for 8 Trainium2 NeuronCores.

Sharding: data-parallel over B*L rows. Core c handles batch b=c//4,
target rows [ (c%4)*512, (c%4+1)*512 ).  All computation is row-local
(edge_i = repeat(arange(L), 8) means each edge scatters back to its own
source row), so there are no collectives; each core needs its target
row-chunk plus the full context of its batch.

v4: the per-edge low-rank einsums are factored through the shared Wl2
weight and run on the TensorEngine instead of DVE:
  Lamraw[e,h,r,:] = sum_k g17[e,k] W2full[k,h,r,:]   (g17 = [gelu .. 1])
  Ld_raw[e,h,r]  = sum_k g17[e,k] Q[e,h,(r,k)],  Q = diff^T-mm-W2T  (PE)
  nrm2[e,h,r]    = sum_k g17[e,k] GG[e,(h,r,k)], GG = g17-mm-G2     (PE)
  y = Ld_raw/nrm2;  C[e->row,h,(k,r)] = sum_w g17 y
  res[row,h,:]   = C^T-mm-W2C (PE) + sum_w alpha_w diff_w
DVE only does the cheap k/w-contractions (1088-wide) instead of the
4096-wide per-edge d/r products+reduces.
"""

import math

import numpy as np

import concourse.bass as bass
import concourse.bacc as bacc
import concourse.tile as tile
from concourse import mybir
from concourse.bass_utils import run_bass_kernel_spmd
from concourse.masks import make_identity

F32 = mybir.dt.float32
BF16 = mybir.dt.bfloat16
U32 = mybir.dt.uint32
AX = mybir.AxisListType
ALU = mybir.AluOpType
ACTF = mybir.ActivationFunctionType

# problem constants (hardcoded per the harness contract)
B, L, K, D = 2, 2048, 2048, 512
H, R, WWIN, T, EH = 8, 8, 8, 2, 16
HD = D // H            # 64
LC = L * B // 8        # 512 rows per core
NT = LC // 128         # 4 l-tiles per core
KT = K // 128          # 16 k-tiles
CROW = D + 2 * EH      # 544: gather-table row [v(512) | ca(16) | cl(16)]
TWO_PI = 2.0 * math.pi
KH = EH + 1            # 17: g dims + folded-bias constant
KR = KH * R            # 136


def build_program():
    nc = bacc.Bacc()

    # ---------------- external I/O ----------------
    tT = nc.dram_tensor("tT", [D, LC], F32, kind="ExternalInput")        # target^T
    cT = nc.dram_tensor("cT", [D, K], F32, kind="ExternalInput")         # context^T
    Wt_d = nc.dram_tensor("Wt", [D, D], F32, kind="ExternalInput")
    Wcb_d = nc.dram_tensor("Wcb", [D, D], BF16, kind="ExternalInput")
    cTb_d = nc.dram_tensor("cTb", [D, K], BF16, kind="ExternalInput")
    Waclb_d = nc.dram_tensor("Waclb", [D, 32], BF16, kind="ExternalInput")
    Wo_d = nc.dram_tensor("Wo", [D, D], F32, kind="ExternalInput")
    bpack_d = nc.dram_tensor("bpack", [128, D], F32, kind="ExternalInput")  # bt@0|bc@32|bo@64
    Wtr3_d = nc.dram_tensor("Wtr3", [D, 48], F32, kind="ExternalInput")  # [Ws1t|Wa1t|Wl1t]
    Ws1c_d = nc.dram_tensor("Ws1c", [D, EH], F32, kind="ExternalInput")
    bs1_d = nc.dram_tensor("bs1", [1, EH], F32, kind="ExternalInput")
    bacl_d = nc.dram_tensor("bacl", [1, 32], F32, kind="ExternalInput")  # [ba1|bl1]
    Ws2bd_d = nc.dram_tensor("Ws2bd", [128, 8], F32, kind="ExternalInput")
    Wa2_d = nc.dram_tensor("Wa2", [1, EH], F32, kind="ExternalInput")
    ba2_d = nc.dram_tensor("ba2", [1, 1], F32, kind="ExternalInput")
    # v4 factored-Wl2 tables
    W2Tp_d = nc.dram_tensor("W2Tp", [128, 4 * 2 * KR], BF16, kind="ExternalInput")
    W2C_d = nc.dram_tensor("W2C", [128, H * HD], BF16, kind="ExternalInput")
    W2C2_d = nc.dram_tensor("W2C2", [8, H * HD], BF16, kind="ExternalInput")
    G2_d = nc.dram_tensor("G2", [128, H * KR], BF16, kind="ExternalInput")
    stp_d = nc.dram_tensor("stp", [128, T * NT], F32, kind="ExternalInput")
    lcol_d = nc.dram_tensor("lcol", [128, NT], F32, kind="ExternalInput")
    invf_d = nc.dram_tensor("invf", [1, HD // 2], F32, kind="ExternalInput")
    y_d = nc.dram_tensor("y", [LC, D], F32, kind="ExternalOutput")

    # internal DRAM gather table
    Tctx = nc.dram_tensor("Tctx", [K, CROW], BF16)

    # ---------------- persistent SBUF (static allocs, before pools) ----------
    ident = nc.alloc_sbuf_tensor("ident", [128, 128], F32).ap()
    identb = nc.alloc_sbuf_tensor("identb", [128, 128], BF16).ap()
    ones1 = nc.alloc_sbuf_tensor("ones1", [128, 512], F32).ap()
    u_sb = [nc.alloc_sbuf_tensor(f"u{i}", [128, D], F32).ap() for i in range(NT)]
    trio = [nc.alloc_sbuf_tensor(f"trio{i}", [128, 48], F32).ap() for i in range(NT)]
    cpTrep = nc.alloc_sbuf_tensor("cpTrep", [128, K], F32).ap()
    tpbT = nc.alloc_sbuf_tensor("tpbT", [128, NT * 16], F32).ap()
    invf_sb = nc.alloc_sbuf_tensor("invfsb", [128, HD // 2], F32).ap()
    wa2_sb = nc.alloc_sbuf_tensor("wa2sb", [128, EH], F32).ap()
    ba2_sb = nc.alloc_sbuf_tensor("ba2sb", [128, 1], F32).ap()
    stp_sb = nc.alloc_sbuf_tensor("stpsb", [128, T * NT], F32).ap()
    stpn_sb = nc.alloc_sbuf_tensor("stpnsb", [128, T * NT], F32).ap()
    lcol_sb = nc.alloc_sbuf_tensor("lcolsb", [128, NT], F32).ap()
    bs1_sb = nc.alloc_sbuf_tensor("bs1sb", [1, EH], F32).ap()
    bacl_sb = nc.alloc_sbuf_tensor("baclsb", [1, 32], F32).ap()
    bpack_sb = nc.alloc_sbuf_tensor("bpacksb", [128, D], F32).ap()
    Ws2bd_sb = nc.alloc_sbuf_tensor("ws2bdsb", [128, 8], F32).ap()
    Wtr3_sb = nc.alloc_sbuf_tensor("wtr3sb", [128, 4 * 48], F32).ap()
    Waclb_sb = nc.alloc_sbuf_tensor("waclsb", [128, 4 * 32], BF16).ap()
    W2Tp_sb = nc.alloc_sbuf_tensor("w2tpsb", [128, 4 * 2 * KR], BF16).ap()
    W2C_sb = nc.alloc_sbuf_tensor("w2csb", [128, H * HD], BF16).ap()
    W2C2_sb = nc.alloc_sbuf_tensor("w2c2sb", [8, H * HD], BF16).ap()
    G2_sb = nc.alloc_sbuf_tensor("g2sb", [128, H * KR], BF16).ap()
    halfpi = nc.alloc_sbuf_tensor("halfpi", [128, 1], F32).ap()
    onec = nc.alloc_sbuf_tensor("onec", [128, 1], F32).ap()

    with tile.TileContext(nc) as tc:
        with (
            tc.tile_pool(name="ld", bufs=2) as ldp,             # small staging tiles
            tc.tile_pool(name="gbp", bufs=2) as gbp,            # gather block
            tc.tile_pool(name="big", bufs=2) as bigp,           # 8704-wide bf16 tiles
            tc.tile_pool(name="med", bufs=2) as medp,
            tc.tile_pool(name="sml", bufs=2) as smlp,
            tc.tile_pool(name="wp", bufs=1) as wp,
            tc.tile_pool(name="psA", bufs=2, space="PSUM") as psA,   # [128,512] f32
            tc.tile_pool(name="psC", bufs=1, space="PSUM") as psC,   # ctx/trio 2nd
            tc.tile_pool(name="psS", bufs=2, space="PSUM") as psS,   # scores/gT4
            tc.tile_pool(name="psT", bufs=2, space="PSUM") as psT,   # transposes/Q
            tc.tile_pool(name="psR", bufs=1, space="PSUM") as psRp,  # res accum
        ):
            # ---------- constants ----------
            make_identity(nc, ident)
            make_identity(nc, identb)
            nc.vector.memset(ones1, 1.0)
            nc.vector.memset(halfpi, math.pi / 2)
            nc.vector.memset(onec, 1.0)
            nc.sync.dma_start(out=invf_sb, in_=invf_d[:].partition_broadcast(128))
            nc.sync.dma_start(out=wa2_sb, in_=Wa2_d[:].partition_broadcast(128))
            nc.sync.dma_start(out=ba2_sb, in_=ba2_d[:].partition_broadcast(128))
            nc.sync.dma_start(out=lcol_sb, in_=lcol_d[:])
            nc.sync.dma_start(out=bs1_sb, in_=bs1_d[:])
            nc.sync.dma_start(out=bacl_sb, in_=bacl_d[:])
            nc.sync.dma_start(out=bpack_sb, in_=bpack_d[:])
            nc.sync.dma_start(out=Ws2bd_sb, in_=Ws2bd_d[:])
            nc.sync.dma_start(out=W2Tp_sb, in_=W2Tp_d[:])
            nc.sync.dma_start(out=W2C_sb, in_=W2C_d[:])
            nc.sync.dma_start(out=W2C2_sb, in_=W2C2_d[:])
            nc.sync.dma_start(out=G2_sb, in_=G2_d[:])
            for dc in range(4):
                sl = slice(dc * 128, (dc + 1) * 128)
                nc.sync.dma_start(out=Wtr3_sb[:, dc * 48:(dc + 1) * 48], in_=Wtr3_d[sl, :])
                nc.sync.dma_start(out=Waclb_sb[:, dc * 32:(dc + 1) * 32], in_=Waclb_d[sl, :])

            bt_b = bpack_sb[0:1, :]
            bc_b = bpack_sb[32:33, :]
            bo_b = bpack_sb[64:65, :]

            def load_w(dram):
                t = wp.tile([128, 4 * D], F32, tag="wrhs")
                for dc in range(4):
                    nc.sync.dma_start(out=t[:, dc * D:(dc + 1) * D],
                                      in_=dram[dc * 128:(dc + 1) * 128, :])
                return t

            def softplus(dst, src, bias_ap, tmp_pool, tmp_tag):
                """dst = softplus(src + bias) = relu(x) + ln(1+exp(-|x|))."""
                shp = [src.shape[0], src.free_size()]
                a = tmp_pool.tile(shp, F32, tag=tmp_tag)
                if bias_ap is None:
                    nc.scalar.activation(a[:], src, ACTF.Abs)
                    nc.vector.tensor_scalar(dst, src, 0.0, scalar2=None, op0=ALU.max)
                else:
                    nc.scalar.activation(a[:], src, ACTF.Abs, bias=bias_ap)
                    nc.vector.tensor_scalar(dst, src, bias_ap, scalar2=0.0,
                                            op0=ALU.add, op1=ALU.max)
                nc.scalar.activation(a[:], a[:], ACTF.Exp, scale=-1.0)
                nc.scalar.activation(a[:], a[:], ACTF.Ln, bias=onec[:, 0:1])
                nc.vector.tensor_tensor(dst, dst, a[:], op=ALU.add)

            stp_raw = smlp.tile([128, T * NT], F32, tag="stpraw")
            nc.sync.dma_start(out=stp_raw[:], in_=stp_d[:])
            softplus(stp_sb, stp_raw[:], None, smlp, "sptmp")
            nc.vector.tensor_scalar_mul(stpn_sb, stp_sb, -1.0)

            # ---------- dense projections ----------
            # cpT [16, K] = Ws1c.T @ context^T + bs1, replicated 8x on partitions
            cpT = cpTrep[0:EH, :]
            for nt4 in range(4):
                nsl = slice(nt4 * 512, (nt4 + 1) * 512)
                ps = psA.tile([128, 512], F32, space="PSUM", tag="mmps")
                nc.tensor.matmul(ps[:EH, :], bs1_sb[:1, :], ones1[:1, :512],
                                 start=True, stop=False)
                for dc in range(4):
                    lh = ldp.tile([128, EH], F32, tag="lhst16", bufs=1)
                    nc.sync.dma_start(out=lh[:],
                                      in_=Ws1c_d[dc * 128:(dc + 1) * 128, :])
                    rh = ldp.tile([128, 512], F32, tag="ctchunk", bufs=2)
                    nc.sync.dma_start(out=rh[:], in_=cT[dc * 128:(dc + 1) * 128, nsl])
                    nc.tensor.matmul(ps[:EH, :], lh[:], rh[:],
                                     start=False, stop=(dc == 3))
                nc.vector.tensor_copy(cpT[:, nsl], ps[:EH, :])
            for ls in range(1, 8):
                nc.sync.dma_start(out=cpTrep[ls * 16:(ls + 1) * 16, :], in_=cpT[:, :])

            # trio projection first (scores need only trio+cpT), then u
            for lt in range(NT):
                pst3 = psC.tile([128, 512], F32, space="PSUM", tag="ctx2")
                for dc in range(4):
                    lh = ldp.tile([128, 128], F32, tag="lhst", bufs=3)
                    nc.sync.dma_start(
                        out=lh[:], in_=tT[dc * 128:(dc + 1) * 128,
                                          lt * 128:(lt + 1) * 128])
                    nc.tensor.matmul(pst3[:, :48], lh[:],
                                     Wtr3_sb[:, dc * 48:(dc + 1) * 48],
                                     start=(dc == 0), stop=(dc == 3))
                nc.scalar.copy(trio[lt][:], pst3[:, :48])
                # tpbT: per-octet score bias columns, partition p = ls*16 + e
                for oc in range(16):
                    nc.sync.dma_start(
                        out=tpbT[:, lt * 16 + oc:lt * 16 + oc + 1],
                        in_=trio[lt][oc * 8:(oc + 1) * 8, 0:EH])
            Wt_t = load_w(Wt_d)
            for lt in range(NT):
                psu = psA.tile([128, 512], F32, space="PSUM", tag="mmps")
                nc.tensor.matmul(psu[:, :], ones1[0:1, :128], bt_b[:1, :],
                                 start=True, stop=False)
                for dc in range(4):
                    lh = ldp.tile([128, 128], F32, tag="lhst", bufs=3)
                    nc.sync.dma_start(
                        out=lh[:], in_=tT[dc * 128:(dc + 1) * 128,
                                          lt * 128:(lt + 1) * 128])
                    nc.tensor.matmul(psu[:, :], lh[:],
                                     Wt_t[:, dc * D:(dc + 1) * D],
                                     start=False, stop=(dc == 3))
                nc.scalar.copy(u_sb[lt][:], psu[:, :])

            # context projection -> Tctx, bf16 single-pass matmuls
            Wcb_t = wp.tile([128, 4 * D], BF16, tag="wrhsb")
            for dc in range(4):
                nc.sync.dma_start(out=Wcb_t[:, dc * D:(dc + 1) * D],
                                  in_=Wcb_d[dc * 128:(dc + 1) * 128, :])

            def emit_ctx_kt(kt):
                psv = psA.tile([128, 512], F32, space="PSUM", tag="mmps")
                psa = psC.tile([128, 512], F32, space="PSUM", tag="ctx2")
                nc.tensor.matmul(psv[:, :], ones1[32:33, :128], bc_b[:1, :],
                                 start=True, stop=False)
                nc.tensor.matmul(psa[:, :32], ones1[0:1, :128], bacl_sb[:1, :],
                                 start=True, stop=False)
                for dc in range(4):
                    lh = ldp.tile([128, 128], BF16, tag="lhstb")
                    nc.sync.dma_start(
                        out=lh[:], in_=cTb_d[dc * 128:(dc + 1) * 128,
                                            kt * 128:(kt + 1) * 128])
                    nc.tensor.matmul(psv[:, :], lh[:],
                                     Wcb_t[:, dc * D:(dc + 1) * D],
                                     start=False, stop=(dc == 3))
                    nc.tensor.matmul(psa[:, :32], lh[:],
                                     Waclb_sb[:, dc * 32:(dc + 1) * 32],
                                     start=False, stop=(dc == 3))
                stg = ldp.tile([128, CROW], BF16, tag="stgb")
                nc.scalar.copy(stg[:, 0:D], psv[:, :])
                nc.scalar.copy(stg[:, D:D + 32], psa[:, :32])
                nc.sync.dma_start(out=Tctx[kt * 128:(kt + 1) * 128, :],
                                  in_=stg[:, :])

            # ---------- score phase ----------
            class ScoreEmitter:
                """Stepwise score emission so octets can interleave into the
                t-loop of the previous l-tile (fills PE/ACT while DVE works)."""

                def __init__(self, lt, filler=None):
                    self.lt = lt
                    self.filler = filler
                    self.oc = 0
                    self.scores = medp.tile([128, K], F32, tag="scores",
                                            bufs=1)

                def step(self, n=1):
                    lt = self.lt
                    for oc in range(self.oc, min(self.oc + n, 16)):
                        if self.filler is not None:
                            self.filler(oc)
                        for hf in range(2):
                            g_sc = medp.tile([128, K // 2], F32, tag="gsc",
                                             bufs=2)
                            nc.scalar.activation(
                                g_sc[:], cpTrep[:, hf * 1024:(hf + 1) * 1024],
                                ACTF.Gelu,
                                bias=tpbT[:, lt * 16 + oc:lt * 16 + oc + 1])
                            for nq in range(2):
                                col = hf * 1024 + nq * 512
                                pssc = psS.tile([8, 512], F32, space="PSUM",
                                                tag="small")
                                nc.tensor.matmul(pssc[:, :], Ws2bd_sb[:],
                                                 g_sc[:, nq * 512:(nq + 1) * 512],
                                                 start=True, stop=True)
                                sstg = medp.tile([8, 512], F32, tag="sstg",
                                                 bufs=2)
                                if oc % 2 == 0:
                                    nc.scalar.copy(sstg[:], pssc[:, :])
                                else:
                                    nc.vector.tensor_copy(sstg[:], pssc[:, :])
                                nc.sync.dma_start(
                                    out=self.scores[oc * 8:(oc + 1) * 8,
                                                    col:col + 512],
                                    in_=sstg[:])
                    self.oc = min(self.oc + n, 16)

                def finish(self):
                    self.step(16 - self.oc)
                    mx8 = smlp.tile([128, 8], F32, tag="mx8")
                    idx = smlp.tile([128, 8], U32, tag="idx", bufs=2)
                    nc.vector.max(out=mx8[:], in_=self.scores[:])
                    nc.vector.max_index(out=idx[:], in_max=mx8[:],
                                        in_values=self.scores[:])
                    return idx

            def emit_gather(idx):
                gb = gbp.tile([128, WWIN * CROW], BF16, tag="gb")
                for w in range(WWIN):
                    nc.gpsimd.indirect_dma_start(
                        out=gb[:, w * CROW:(w + 1) * CROW],
                        out_offset=None,
                        in_=Tctx[:, :],
                        in_offset=bass.IndirectOffsetOnAxis(ap=idx[:, w:w + 1],
                                                            axis=0),
                    )
                return gb

            # score-0 interleaves the Tctx context projection as filler
            em0 = ScoreEmitter(0, filler=emit_ctx_kt)
            idx_next = em0.finish()
            gb_next = emit_gather(idx_next)

            # ---------- per l-tile ----------
            for lt in range(NT):
                idx = idx_next
                gb = gb_next
                gbv = gb[:].rearrange("p (w c) -> p w c", w=8)
                # next l-tile's scores drip-fed into this tile's t-loop
                em = ScoreEmitter(lt + 1) if lt + 1 < NT else None

                # ----- per-edge angles -----
                jf = smlp.tile([128, 8], F32, tag="jf")
                nc.vector.tensor_copy(jf[:], idx[:])
                delta = smlp.tile([128, 8], F32, tag="delta")
                nc.vector.tensor_scalar(delta[:], jf[:], lcol_sb[:, lt:lt + 1],
                                        scalar2=None, op0=ALU.subtract)
                ang = medp.tile([128, 8 * 32], F32, tag="ang", bufs=1)
                nc.vector.tensor_tensor(
                    out=ang[:].rearrange("p (w f) -> p w f", w=8),
                    in0=delta[:].unsqueeze(2).to_broadcast((128, 8, 32)),
                    in1=invf_sb[:].unsqueeze(1).to_broadcast((128, 8, 32)),
                    op=ALU.mult)
                # range-reduce to [-pi, pi]
                MAGIC = 1.5 * 2.0 ** 23
                angt = medp.tile([128, 8 * 32], F32, tag="angt", bufs=1)
                nc.vector.tensor_scalar_mul(angt[:], ang[:], 1.0 / TWO_PI)
                angr = medp.tile([128, 8 * 32], F32, tag="angr", bufs=1)
                nc.vector.tensor_scalar(angr[:], angt[:], MAGIC, scalar2=MAGIC,
                                        op0=ALU.add, op1=ALU.subtract)
                nc.vector.tensor_sub(angt[:], angt[:], angr[:])
                nc.vector.tensor_scalar_mul(ang[:], angt[:], TWO_PI)
                # sinb = sin(-ang) = sin(ref-ang); cosb = cos via sin(pi/2-|ang|)
                cosb = medp.tile([128, 8 * 32], BF16, tag="cosb")
                sinb = medp.tile([128, 8 * 32], BF16, tag="sinb")
                nc.scalar.activation(sinb[:], ang[:], ACTF.Sin, scale=-1.0)
                nc.vector.tensor_scalar_mul(angr[:], ang[:], -1.0)
                nc.vector.tensor_max(angt[:], ang[:], angr[:])
                nc.scalar.activation(cosb[:], angt[:], ACTF.Sin, scale=-1.0,
                                     bias=halfpi[:, 0:1])

                # ----- alphas = softplus(gelu(ta+ca) @ Wa2 + ba2) -----
                ha = smlp.tile([128, 8 * EH], F32, tag="ha")
                nc.vector.tensor_tensor(
                    out=ha[:].rearrange("p (w c) -> p w c", w=8),
                    in0=trio[lt][:, 16:32].unsqueeze(1).to_broadcast((128, 8, EH)),
                    in1=gbv[:, :, D:D + EH],
                    op=ALU.add)
                nc.scalar.activation(ha[:], ha[:], ACTF.Gelu)
                haw = smlp.tile([128, 8 * EH], F32, tag="haw")
                nc.vector.tensor_tensor(
                    out=haw[:].rearrange("p (w c) -> p w c", w=8),
                    in0=ha[:].rearrange("p (w c) -> p w c", w=8),
                    in1=wa2_sb[:].unsqueeze(1).to_broadcast((128, 8, EH)),
                    op=ALU.mult)
                alphas = smlp.tile([128, 8], F32, tag="alphas")
                nc.vector.tensor_reduce(alphas[:], haw[:].rearrange(
                    "p (w c) -> p w c", w=8), axis=AX.X, op=ALU.add)
                softplus(alphas[:], alphas[:], ba2_sb[:, 0:1], smlp, "sptmp")
                alphab = smlp.tile([128, 8], BF16, tag="alphab")
                nc.scalar.copy(alphab[:], alphas[:])

                # ----- g17 = [gelu(tl + cl) | 1], per-w transposes -----
                gmat17 = smlp.tile([128, 8 * KH], F32, tag="gmat17", bufs=1)
                g3 = gmat17[:].rearrange("p (w c) -> p w c", w=8)
                # gelu(tl + cl): add then gelu
                nc.vector.tensor_tensor(
                    out=g3[:, :, 0:EH],
                    in0=trio[lt][:, 32:48].unsqueeze(1).to_broadcast((128, 8, EH)),
                    in1=gbv[:, :, D + EH:D + 2 * EH],
                    op=ALU.add)
                nc.scalar.activation(g3[:, :, 0:EH], g3[:, :, 0:EH], ACTF.Gelu)
                nc.vector.memset(g3[:, :, EH:KH], 1.0)

                # gT4 [4 sectors x 17 rows, 2 quads x 128]: g17^T per w
                gT4 = gbp.tile([128, 2 * 128], BF16, tag="gT4")
                nc.vector.memset(gT4[:], 0.0)
                for w in range(WWIN):
                    q, s = w // 4, w % 4
                    pst = psS.tile([KH, 128], F32, space="PSUM", tag="small")
                    nc.tensor.transpose(
                        out=pst[:, :], in_=g3[:, w, :], identity=ident)
                    nc.vector.tensor_copy(
                        gT4[32 * s:32 * s + KH, q * 128:(q + 1) * 128], pst[:, :])

                # grep [p, (w r k)] bf16; gkw [p, (k w)] bf16
                grep = smlp.tile([128, WWIN * KR], BF16, tag="grep", bufs=2)
                nc.vector.tensor_copy(
                    grep[:].rearrange("p (w r k) -> p w r k", w=8, r=R),
                    g3.unsqueeze(2).to_broadcast((128, 8, R, KH)))
                gkw = smlp.tile([128, KH * WWIN], BF16, tag="gkw", bufs=2)
                nc.vector.tensor_copy(
                    gkw[:].rearrange("p (k w) -> p k w", k=KH),
                    gmat17[:].rearrange("p (w k) -> p k w", w=8))

                # ----- norms: GG = g17 @ G2 (PE), nrm2 = g17 . GG (DVE) -----
                GGsb = bigp.tile([128, WWIN * H * KR], BF16, tag="big")
                for w in range(WWIN):
                    q, s = w // 4, w % 4
                    for ch, (c0, cw) in enumerate(((0, 512), (512, 512), (1024, 64))):
                        psg = psA.tile([128, 512], F32, space="PSUM", tag="mmps")
                        nc.tensor.matmul(
                            psg[:, :cw],
                            gT4[32 * s:32 * s + 32, q * 128:(q + 1) * 128],
                            G2_sb[32 * s:32 * s + 32, c0:c0 + cw],
                            start=True, stop=True, tile_position=(32 * s, 0))
                        nc.scalar.copy(
                            GGsb[:, w * H * KR + c0:w * H * KR + c0 + cw],
                            psg[:, :cw])
                nrm2 = smlp.tile([128, WWIN * H * R], F32, tag="nrm2", bufs=1)
                n3 = nrm2[:].rearrange("p (w h r) -> p w h r", w=8, h=H)
                for w in range(WWIN):
                    gg_w = GGsb[:].rearrange(
                        "p (w hr k) -> p w hr k", w=8, k=KH)[:, w]
                    nc.vector.tensor_tensor(
                        out=gg_w,
                        in0=gg_w,
                        in1=grep[:].rearrange(
                            "p (w r k) -> p w r k", w=8, r=R)[:, w, 0:1, :]
                            .to_broadcast((128, H * R, KH)),
                        op=ALU.mult)
                    nc.vector.tensor_reduce(
                        n3[:, w], gg_w, axis=AX.X, op=ALU.add)
                rec = smlp.tile([128, WWIN * H * R], F32, tag="rec", bufs=2)
                nc.vector.tensor_scalar_max(rec[:], nrm2[:], 1e-24)
                nc.vector.reciprocal(rec[:], rec[:])
                # rec viewed (h, r, w) for the y-scale
                recv = rec[:].rearrange("p (w h r) -> p h r w", w=8, h=H)

                # ----- t-loop -----
                for t in range(T):
                    stc = slice(t * NT + lt, t * NT + lt + 1)
                    if t == T - 1 and em is not None:
                        # next tile's topk + gather hide under this t-pass
                        idx_next = em.finish()
                        gb_next = emit_gather(idx_next)
                    # u casts: ub = bf16(u); ubrot = rot_half(u) = [-u_hi, u_lo]
                    ub = medp.tile([128, D], BF16, tag="ub", bufs=2)
                    nc.scalar.copy(ub[:], u_sb[lt][:])
                    ubrot = medp.tile([128, D], BF16, tag="ubrot", bufs=2)
                    ur3 = ubrot[:].rearrange("p (h a b) -> p h a b", h=H, a=2)
                    uv3 = u_sb[lt][:].rearrange("p (h a b) -> p h a b", h=H, a=2)
                    nc.scalar.activation(ur3[:, :, 0, :], uv3[:, :, 1, :],
                                         ACTF.Copy, scale=-1.0)
                    nc.scalar.copy(ur3[:, :, 1, :], uv3[:, :, 0, :])

                    # diff [p, (w h d)] bf16
                    t0 = medp.tile([128, WWIN * D], BF16, tag="t0", bufs=1)
                    nc.vector.tensor_tensor(
                        out=t0[:].rearrange("p (w g b) -> p w g b", w=8, g=2 * H),
                        in0=ub[:].rearrange("p (g b) -> p g b", g=2 * H)
                            .unsqueeze(1).to_broadcast((128, 8, 2 * H, 32)),
                        in1=cosb[:].rearrange("p (w f) -> p w f", w=8)
                            .unsqueeze(2).to_broadcast((128, 8, 2 * H, 32)),
                        op=ALU.mult)
                    t1 = medp.tile([128, WWIN * D], BF16, tag="t1", bufs=1)
                    nc.vector.tensor_tensor(
                        out=t1[:].rearrange("p (w g b) -> p w g b", w=8, g=2 * H),
                        in0=ubrot[:].rearrange("p (g b) -> p g b", g=2 * H)
                            .unsqueeze(1).to_broadcast((128, 8, 2 * H, 32)),
                        in1=sinb[:].rearrange("p (w f) -> p w f", w=8)
                            .unsqueeze(2).to_broadcast((128, 8, 2 * H, 32)),
                        op=ALU.mult)
                    nc.vector.tensor_tensor(out=t0[:], in0=t0[:], in1=t1[:],
                                            op=ALU.add)
                    diff = medp.tile([128, WWIN * D], BF16, tag="diff", bufs=1)
                    nc.vector.tensor_tensor(
                        out=diff[:].rearrange("p (w e) -> p w e", w=8),
                        in0=t0[:].rearrange("p (w e) -> p w e", w=8),
                        in1=gbv[:, :, 0:D],
                        op=ALU.subtract)

                    # alpha term early (DVE busy while PE transposes)
                    adiff = medp.tile([128, WWIN * D], BF16, tag="adiff", bufs=1)
                    nc.vector.tensor_tensor(
                        out=adiff[:].rearrange("p (w e) -> p w e", w=8),
                        in0=diff[:].rearrange("p (w e) -> p w e", w=8),
                        in1=alphab[:].unsqueeze(2).to_broadcast((128, 8, D)),
                        op=ALU.mult)
                    rsa = smlp.tile([128, D], F32, tag="rsa", bufs=1)
                    nc.vector.tensor_reduce(
                        rsa[:],
                        adiff[:].rearrange("p (w e) -> p e w", w=8),
                        axis=AX.X, op=ALU.add)

                    # diff^T per (w, head-pair): [128,(128)] -> [128 rows of (h2 d)]
                    dT = medp.tile([128, WWIN * D], BF16, tag="dT", bufs=1)
                    d4 = dT[:].rearrange("p (w hp e) -> p w hp e", w=8, hp=4)
                    dv4 = diff[:].rearrange("p (w hp e) -> p w hp e", w=8, hp=4)
                    for w in range(WWIN):
                        for hp in range(4):
                            pst = psT.tile([128, 128], BF16, space="PSUM",
                                           tag="tq")
                            nc.tensor.transpose(
                                out=pst[:, :], in_=dv4[:, w, hp, :],
                                identity=identb)
                            nc.scalar.copy(d4[:, w, hp, :], pst[:, :])

                    if em is not None and t == 0:
                        em.step(4)

                    # Q pair matmuls: psQ[row, 2*KR] per (w, hp)
                    Qsb = bigp.tile([128, H * WWIN * KR], BF16, tag="big")
                    q4 = Qsb[:].rearrange("p (h w rk) -> p h w rk", h=H, w=8)
                    for w in range(WWIN):
                        for hp in range(4):
                            psq = psT.tile([128, 2 * KR], F32, space="PSUM",
                                           tag="tq")
                            nc.tensor.matmul(
                                psq[:, :], d4[:, w, hp, :],
                                W2Tp_sb[:, hp * 2 * KR:(hp + 1) * 2 * KR],
                                start=True, stop=True)
                            # evac both heads of the pair
                            nc.scalar.copy(q4[:, 2 * hp, w, :], psq[:, 0:KR])
                            nc.scalar.copy(q4[:, 2 * hp + 1, w, :],
                                           psq[:, KR:2 * KR])

                    # filler: next tile's score octets run on PE/ACT while
                    # the einsum DVE phase below executes
                    if em is not None and t == 0:
                        em.step(4)

                    # einsum1: y[h,r,w] = sum_k g17 * Q  (normalized by rec)
                    # in-place product into Qsb, then segmented reduce over k
                    nc.vector.tensor_tensor(
                        out=Qsb[:].rearrange("p (h wrk) -> p h wrk", h=H),
                        in0=Qsb[:].rearrange("p (h wrk) -> p h wrk", h=H),
                        in1=grep[:].unsqueeze(1).to_broadcast(
                            (128, H, WWIN * KR)),
                        op=ALU.mult)
                    yt = smlp.tile([128, H * R * WWIN], F32, tag="yt", bufs=1)
                    yv = yt[:].rearrange("p (h r w) -> p h w r", h=H, r=R)
                    nc.vector.tensor_reduce(
                        yv,
                        Qsb[:].rearrange("p (hwr k) -> p hwr k", k=KH),
                        axis=AX.X, op=ALU.add)
                    ycast = smlp.tile([128, H * R * WWIN], BF16, tag="ycast", bufs=1)
                    nc.vector.tensor_tensor(
                        out=ycast[:].rearrange("p (h r w) -> p h r w",
                                               h=H, r=R),
                        in0=yt[:].rearrange("p (h r w) -> p h r w", h=H, r=R),
                        in1=recv,
                        op=ALU.mult)

                    # C[h, (k r)] = sum_w g17[w,k] y[h,w,r]
                    C = smlp.tile([128, H * KR], F32, tag="C", bufs=1)
                    c3 = C[:].rearrange("p (h k r) -> p h k r", h=H, k=KH)
                    yc3 = ycast[:].rearrange("p (h r w) -> p h r w", h=H, r=R)
                    gkw3 = gkw[:].rearrange("p (k w) -> p k w", k=KH)
                    for h in range(H):
                        prodC = medp.tile([128, KH * R * WWIN], BF16,
                                          tag="prodC", bufs=2)
                        nc.gpsimd.tensor_tensor(
                            out=prodC[:].rearrange("p (k r w) -> p k r w",
                                                   k=KH, r=R),
                            in0=yc3[:, h].unsqueeze(1).to_broadcast(
                                (128, KH, R, WWIN)),
                            in1=gkw3.unsqueeze(2).to_broadcast(
                                (128, KH, R, WWIN)),
                            op=ALU.mult)
                        nc.vector.tensor_reduce(
                            c3[:, h],
                            prodC[:].rearrange("p (kr w) -> p kr w", w=8),
                            axis=AX.X, op=ALU.add)

                    # filler: more score octets ahead of the CT/res PE phase
                    if em is not None and t == 0:
                        em.step(4)

                    # C^T via PE; res = C^T-mm-W2C + bias part
                    psR = psRp.tile([128, D], F32, space="PSUM", tag="rps")
                    CTm = medp.tile([128, H * 128], BF16, tag="CTm", bufs=1)
                    CTb = medp.tile([8, H * 128], BF16, tag="CTb", bufs=1)
                    for h in range(H):
                        pct = psT.tile([128, 128], F32, space="PSUM", tag="tq")
                        nc.tensor.transpose(
                            out=pct[:, :], in_=C[:, h * KR:h * KR + 128],
                            identity=ident)
                        nc.scalar.copy(CTm[:, h * 128:(h + 1) * 128], pct[:, :])
                        pcb = psT.tile([8, 128], F32, space="PSUM", tag="tq")
                        nc.tensor.transpose(
                            out=pcb[:, :], in_=C[:, h * KR + 128:(h + 1) * KR],
                            identity=ident)
                        nc.scalar.copy(CTb[:, h * 128:(h + 1) * 128], pcb[:, :])
                    for h in range(H):
                        nc.tensor.matmul(
                            psR[:, h * HD:(h + 1) * HD],
                            CTm[:, h * 128:(h + 1) * 128],
                            W2C_sb[:, h * HD:(h + 1) * HD],
                            start=True, stop=False)
                        nc.tensor.matmul(
                            psR[:, h * HD:(h + 1) * HD],
                            CTb[0:8, h * 128:(h + 1) * 128],
                            W2C2_sb[0:8, h * HD:(h + 1) * HD],
                            start=False, stop=True)

                    # u -= step * (res + rsa)
                    tmpu = smlp.tile([128, D], F32, tag="tmpu", bufs=1)
                    nc.vector.tensor_tensor(tmpu[:], psR[:, :], rsa[:],
                                            op=ALU.add)
                    nc.vector.scalar_tensor_tensor(
                        out=u_sb[lt][:], in0=tmpu[:], scalar=stpn_sb[:, stc],
                        in1=u_sb[lt][:], op0=ALU.mult, op1=ALU.add)


            # ---------- output projection: y = u @ Wo + bo ----------
            Wo_t = load_w(Wo_d)
            for lt in range(NT):
                psy = psA.tile([128, 512], F32, space="PSUM", tag="mmps")
                nc.tensor.matmul(psy[:, :], ones1[64:65, :128], bo_b[:1, :],
                                 start=True, stop=False)
                for dc in range(4):
                    pst = psS.tile([128, 128], F32, space="PSUM", tag="small")
                    nc.tensor.transpose(
                        out=pst[:, :], in_=u_sb[lt][:, dc * 128:(dc + 1) * 128],
                        identity=ident)
                    uT = ldp.tile([128, 128], F32, tag="uT", bufs=1)
                    nc.scalar.copy(uT[:], pst[:, :])
                    nc.tensor.matmul(psy[:, :], uT[:], Wo_t[:, dc * D:(dc + 1) * D],
                                     start=False, stop=(dc == 3))
                ystg = ldp.tile([128, 512], F32, tag="stg", bufs=1)
                nc.scalar.copy(ystg[:], psy[:, :])
                nc.sync.dma_start(out=y_d[lt * 128:(lt + 1) * 128, :], in_=ystg[:])

    nc.finalize()
    return nc


def make_in_maps(inputs):
    """Host-side prep: slice/transpose inputs into the 8 per-core input maps."""
    target = np.asarray(inputs["target"], np.float32)
    context = np.asarray(inputs["context"], np.float32)
    Wt = np.asarray(inputs["Wt"], np.float32)
    bt = np.asarray(inputs["bt"], np.float32)
    Wc = np.asarray(inputs["Wc"], np.float32)
    bc = np.asarray(inputs["bc"], np.float32)
    Ws1 = np.asarray(inputs["Ws1"], np.float32)
    bs1 = np.asarray(inputs["bs1"], np.float32)
    Ws2 = np.asarray(inputs["Ws2"], np.float32)
    Wa1 = np.asarray(inputs["Wa1"], np.float32)
    ba1 = np.asarray(inputs["ba1"], np.float32)
    Wa2 = np.asarray(inputs["Wa2"], np.float32)
    ba2 = np.asarray(inputs["ba2"], np.float32)
    Wl1 = np.asarray(inputs["Wl1"], np.float32)
    bl1 = np.asarray(inputs["bl1"], np.float32)
    Wl2 = np.asarray(inputs["Wl2"], np.float32)
    bl2 = np.asarray(inputs["bl2"], np.float32)
    step_sizes = np.asarray(inputs["step_sizes"], np.float32)
    Wo = np.asarray(inputs["Wo"], np.float32)
    bo = np.asarray(inputs["bo"], np.float32)

    import ml_dtypes
    Ws2bd = np.zeros((128, 8), np.float32)
    for ls in range(8):
        Ws2bd[ls * 16:(ls + 1) * 16, ls] = Ws2[:, 0]

    # v4 factored tables
    W2full = np.zeros((KH, H, R, HD), np.float32)
    W2full[:16] = Wl2.reshape(EH, H, R, HD)
    W2full[16] = bl2.reshape(H, R, HD)
    # W2Tp [128, 4*2*KR]: rows (a,d); cols (hp, b, r, k); block-diag in (a==b)
    W2Tp = np.zeros((128, 4 * 2 * KR), np.float32)
    for hp in range(4):
        for bb in range(2):
            h = 2 * hp + bb
            # [d, r, k] table for head h
            tbl = W2full[:, h].transpose(2, 1, 0).reshape(HD, KR)  # d,(r,k)
            W2Tp[bb * HD:(bb + 1) * HD,
                 hp * 2 * KR + bb * KR:hp * 2 * KR + (bb + 1) * KR] = tbl
    # W2C [128, H*HD]: rows (k<16)*8+r; cols h*64+d
    W2C = np.zeros((128, H * HD), np.float32)
    for k in range(EH):
        for r in range(R):
            W2C[k * 8 + r] = W2full[k, :, r, :].reshape(H * HD)
    # W2C2 [8, H*HD]: rows r; cols h*64+d (k=16 bias part)
    W2C2 = np.zeros((8, H * HD), np.float32)
    for r in range(R):
        W2C2[r] = W2full[16, :, r, :].reshape(H * HD)
    # G2 [128, H*KR]: rows (sector s)*32 + k'; cols h*KR + r*KH + k
    G2f = np.einsum("ahrd,bhrd->abhr", W2full, W2full)  # [17,17,H,R]
    G2s = np.zeros((32, H * KR), np.float32)
    G2s[:KH] = G2f.transpose(0, 2, 3, 1).reshape(KH, H * R * KH)
    G2 = np.tile(G2s, (4, 1))

    invf = (1.0 / (10000.0 ** (np.arange(0, HD, 2, dtype=np.float32) / HD)))[None, :]
    bpack = np.zeros((128, D), np.float32)
    bpack[0] = bt
    bpack[32] = bc
    bpack[64] = bo

    common = dict(
        Wt=Wt, Wcb=Wc.astype(ml_dtypes.bfloat16), Wo=Wo, bpack=bpack,
        Wtr3=np.ascontiguousarray(np.concatenate([Ws1[:D], Wa1[:D], Wl1[:D]], axis=1)),
        Ws1c=np.ascontiguousarray(Ws1[D:]),
        Waclb=np.ascontiguousarray(
            np.concatenate([Wa1[D:], Wl1[D:]], axis=1)).astype(ml_dtypes.bfloat16),
        bs1=bs1[None, :],
        bacl=np.concatenate([ba1, bl1])[None, :],
        Ws2bd=Ws2bd, Wa2=np.ascontiguousarray(Wa2.T),
        ba2=np.asarray(ba2, np.float32).reshape(1, 1),
        W2Tp=W2Tp.astype(ml_dtypes.bfloat16),
        W2C=W2C.astype(ml_dtypes.bfloat16),
        W2C2=W2C2.astype(ml_dtypes.bfloat16),
        G2=G2.astype(ml_dtypes.bfloat16),
        invf=np.ascontiguousarray(invf, np.float32),
    )

    in_maps = []
    for c in range(8):
        b, rc = c // 4, c % 4
        rows = slice(rc * LC, (rc + 1) * LC)
        stp = np.ascontiguousarray(
            step_sizes[:, rows].reshape(T, NT, 128).transpose(2, 0, 1)
            .reshape(128, T * NT))
        lcol = np.ascontiguousarray(
            (rc * LC + np.arange(LC, dtype=np.float32)).reshape(NT, 128).T)
        m = dict(common)
        cTf = np.ascontiguousarray(context[b].T)
        m.update(
            tT=np.ascontiguousarray(target[b, rows].T),
            cT=cTf, cTb=cTf.astype(ml_dtypes.bfloat16),
            stp=stp, lcol=lcol,
        )
        in_maps.append(m)
    return in_maps


_NC_CACHE = {}


def kernel(**inputs):
    if "nc" not in _NC_CACHE:
        _NC_CACHE["nc"] = build_program()
    nc = _NC_CACHE["nc"]
    in_maps = make_in_maps(inputs)
    res = run_bass_kernel_spmd(nc, in_maps, list(range(8)))
    out = np.empty((B, L, D), np.float32)
    for c in range(8):
        b, rc = c // 4, c % 4
        out[b, rc * LC:(rc + 1) * LC] = res.results[c]["y"]
    return out


# revision 22
# speedup vs baseline: 1.0018x; 1.0018x over previous
"""CrossConsensus kernel for 8 Trainium2 NeuronCores.

Sharding: data-parallel over B*L rows. Core c handles batch b=c//4,
target rows [ (c%4)*512, (c%4+1)*512 ).  All computation is row-local
(edge_i = repeat(arange(L), 8) means each edge scatters back to its own
source row), so there are no collectives; each core needs its target
row-chunk plus the full context of its batch.

v4: the per-edge low-rank einsums are factored through the shared Wl2
weight and run on the TensorEngine instead of DVE:
  Lamraw[e,h,r,:] = sum_k g17[e,k] W2full[k,h,r,:]   (g17 = [gelu .. 1])
  Ld_raw[e,h,r]  = sum_k g17[e,k] Q[e,h,(r,k)],  Q = diff^T-mm-W2T  (PE)
  nrm2[e,h,r]    = sum_k g17[e,k] GG[e,(h,r,k)], GG = g17-mm-G2     (PE)
  y = Ld_raw/nrm2;  C[e->row,h,(k,r)] = sum_w g17 y
  res[row,h,:]   = C^T-mm-W2C (PE) + sum_w alpha_w diff_w
DVE only does the cheap k/w-contractions (1088-wide) instead of the
4096-wide per-edge d/r products+reduces.
"""

import math

import numpy as np

import concourse.bass as bass
import concourse.bacc as bacc
import concourse.tile as tile
from concourse import mybir
from concourse.bass_utils import run_bass_kernel_spmd
from concourse.masks import make_identity

F32 = mybir.dt.float32
BF16 = mybir.dt.bfloat16
U32 = mybir.dt.uint32
AX = mybir.AxisListType
ALU = mybir.AluOpType
ACTF = mybir.ActivationFunctionType

# problem constants (hardcoded per the harness contract)
B, L, K, D = 2, 2048, 2048, 512
H, R, WWIN, T, EH = 8, 8, 8, 2, 16
HD = D // H            # 64
LC = L * B // 8        # 512 rows per core
NT = LC // 128         # 4 l-tiles per core
KT = K // 128          # 16 k-tiles
CROW = D + 2 * EH      # 544: gather-table row [v(512) | ca(16) | cl(16)]
TWO_PI = 2.0 * math.pi
KH = EH + 1            # 17: g dims + folded-bias constant
KR = KH * R            # 136


def build_program():
    nc = bacc.Bacc()

    # ---------------- external I/O ----------------
    tT = nc.dram_tensor("tT", [D, LC], F32, kind="ExternalInput")        # target^T
    cT = nc.dram_tensor("cT", [D, K], F32, kind="ExternalInput")         # context^T
    Wt_d = nc.dram_tensor("Wt", [D, D], F32, kind="ExternalInput")
    Wcb_d = nc.dram_tensor("Wcb", [D, D], BF16, kind="ExternalInput")
    cTb_d = nc.dram_tensor("cTb", [D, K], BF16, kind="ExternalInput")
    Waclb_d = nc.dram_tensor("Waclb", [D, 32], BF16, kind="ExternalInput")
    Wo_d = nc.dram_tensor("Wo", [D, D], F32, kind="ExternalInput")
    bpack_d = nc.dram_tensor("bpack", [128, D], F32, kind="ExternalInput")  # bt@0|bc@32|bo@64
    Wtr3_d = nc.dram_tensor("Wtr3", [D, 48], F32, kind="ExternalInput")  # [Ws1t|Wa1t|Wl1t]
    Ws1c_d = nc.dram_tensor("Ws1c", [D, EH], F32, kind="ExternalInput")
    bs1_d = nc.dram_tensor("bs1", [1, EH], F32, kind="ExternalInput")
    bacl_d = nc.dram_tensor("bacl", [1, 32], F32, kind="ExternalInput")  # [ba1|bl1]
    Ws2bd_d = nc.dram_tensor("Ws2bd", [128, 8], F32, kind="ExternalInput")
    Wa2_d = nc.dram_tensor("Wa2", [1, EH], F32, kind="ExternalInput")
    ba2_d = nc.dram_tensor("ba2", [1, 1], F32, kind="ExternalInput")
    # v4 factored-Wl2 tables
    W2Tp_d = nc.dram_tensor("W2Tp", [128, 4 * 2 * KR], BF16, kind="ExternalInput")
    W2C_d = nc.dram_tensor("W2C", [128, H * HD], BF16, kind="ExternalInput")
    W2C2_d = nc.dram_tensor("W2C2", [8, H * HD], BF16, kind="ExternalInput")
    G2_d = nc.dram_tensor("G2", [128, H * KR], BF16, kind="ExternalInput")
    stp_d = nc.dram_tensor("stp", [128, T * NT], F32, kind="ExternalInput")
    lcol_d = nc.dram_tensor("lcol", [128, NT], F32, kind="ExternalInput")
    invf_d = nc.dram_tensor("invf", [1, HD // 2], F32, kind="ExternalInput")
    y_d = nc.dram_tensor("y", [LC, D], F32, kind="ExternalOutput")

    # internal DRAM gather table
    Tctx = nc.dram_tensor("Tctx", [K, CROW], BF16)

    # ---------------- persistent SBUF (static allocs, before pools) ----------
    ident = nc.alloc_sbuf_tensor("ident", [128, 128], F32).ap()
    identb = nc.alloc_sbuf_tensor("identb", [128, 128], BF16).ap()
    ones1 = nc.alloc_sbuf_tensor("ones1", [128, 512], F32).ap()
    u_sb = [nc.alloc_sbuf_tensor(f"u{i}", [128, D], F32).ap() for i in range(NT)]
    trio = [nc.alloc_sbuf_tensor(f"trio{i}", [128, 48], F32).ap() for i in range(NT)]
    cpTrep = nc.alloc_sbuf_tensor("cpTrep", [128, K], F32).ap()
    tpbT = nc.alloc_sbuf_tensor("tpbT", [128, NT * 16], F32).ap()
    invf_sb = nc.alloc_sbuf_tensor("invfsb", [128, HD // 2], F32).ap()
    wa2_sb = nc.alloc_sbuf_tensor("wa2sb", [128, EH], F32).ap()
    ba2_sb = nc.alloc_sbuf_tensor("ba2sb", [128, 1], F32).ap()
    stp_sb = nc.alloc_sbuf_tensor("stpsb", [128, T * NT], F32).ap()
    stpn_sb = nc.alloc_sbuf_tensor("stpnsb", [128, T * NT], F32).ap()
    lcol_sb = nc.alloc_sbuf_tensor("lcolsb", [128, NT], F32).ap()
    bs1_sb = nc.alloc_sbuf_tensor("bs1sb", [1, EH], F32).ap()
    bacl_sb = nc.alloc_sbuf_tensor("baclsb", [1, 32], F32).ap()
    bpack_sb = nc.alloc_sbuf_tensor("bpacksb", [128, D], F32).ap()
    Ws2bd_sb = nc.alloc_sbuf_tensor("ws2bdsb", [128, 8], F32).ap()
    Wtr3_sb = nc.alloc_sbuf_tensor("wtr3sb", [128, 4 * 48], F32).ap()
    Waclb_sb = nc.alloc_sbuf_tensor("waclsb", [128, 4 * 32], BF16).ap()
    W2Tp_sb = nc.alloc_sbuf_tensor("w2tpsb", [128, 4 * 2 * KR], BF16).ap()
    W2C_sb = nc.alloc_sbuf_tensor("w2csb", [128, H * HD], BF16).ap()
    W2C2_sb = nc.alloc_sbuf_tensor("w2c2sb", [8, H * HD], BF16).ap()
    G2_sb = nc.alloc_sbuf_tensor("g2sb", [128, H * KR], BF16).ap()
    halfpi = nc.alloc_sbuf_tensor("halfpi", [128, 1], F32).ap()
    onec = nc.alloc_sbuf_tensor("onec", [128, 1], F32).ap()

    with tile.TileContext(nc) as tc:
        with (
            tc.tile_pool(name="ld", bufs=2) as ldp,             # small staging tiles
            tc.tile_pool(name="gbp", bufs=2) as gbp,            # gather block
            tc.tile_pool(name="big", bufs=2) as bigp,           # 8704-wide bf16 tiles
            tc.tile_pool(name="med", bufs=2) as medp,
            tc.tile_pool(name="sml", bufs=2) as smlp,
            tc.tile_pool(name="wp", bufs=1) as wp,
            tc.tile_pool(name="psA", bufs=2, space="PSUM") as psA,   # [128,512] f32
            tc.tile_pool(name="psC", bufs=1, space="PSUM") as psC,   # ctx/trio 2nd
            tc.tile_pool(name="psS", bufs=2, space="PSUM") as psS,   # scores/gT4
            tc.tile_pool(name="psT", bufs=2, space="PSUM") as psT,   # transposes/Q
            tc.tile_pool(name="psR", bufs=1, space="PSUM") as psRp,  # res accum
        ):
            # ---------- constants ----------
            make_identity(nc, ident)
            make_identity(nc, identb)
            nc.vector.memset(ones1, 1.0)
            nc.vector.memset(halfpi, math.pi / 2)
            nc.vector.memset(onec, 1.0)
            nc.sync.dma_start(out=invf_sb, in_=invf_d[:].partition_broadcast(128))
            nc.sync.dma_start(out=wa2_sb, in_=Wa2_d[:].partition_broadcast(128))
            nc.sync.dma_start(out=ba2_sb, in_=ba2_d[:].partition_broadcast(128))
            nc.sync.dma_start(out=lcol_sb, in_=lcol_d[:])
            nc.sync.dma_start(out=bs1_sb, in_=bs1_d[:])
            nc.sync.dma_start(out=bacl_sb, in_=bacl_d[:])
            nc.sync.dma_start(out=bpack_sb, in_=bpack_d[:])
            nc.sync.dma_start(out=Ws2bd_sb, in_=Ws2bd_d[:])
            nc.sync.dma_start(out=W2Tp_sb, in_=W2Tp_d[:])
            nc.sync.dma_start(out=W2C_sb, in_=W2C_d[:])
            nc.sync.dma_start(out=W2C2_sb, in_=W2C2_d[:])
            nc.sync.dma_start(out=G2_sb, in_=G2_d[:])
            for dc in range(4):
                sl = slice(dc * 128, (dc + 1) * 128)
                nc.sync.dma_start(out=Wtr3_sb[:, dc * 48:(dc + 1) * 48], in_=Wtr3_d[sl, :])
                nc.sync.dma_start(out=Waclb_sb[:, dc * 32:(dc + 1) * 32], in_=Waclb_d[sl, :])

            bt_b = bpack_sb[0:1, :]
            bc_b = bpack_sb[32:33, :]
            bo_b = bpack_sb[64:65, :]

            def load_w(dram):
                t = wp.tile([128, 4 * D], F32, tag="wrhs")
                for dc in range(4):
                    nc.sync.dma_start(out=t[:, dc * D:(dc + 1) * D],
                                      in_=dram[dc * 128:(dc + 1) * 128, :])
                return t

            def softplus(dst, src, bias_ap, tmp_pool, tmp_tag):
                """dst = softplus(src + bias) = relu(x) + ln(1+exp(-|x|))."""
                shp = [src.shape[0], src.free_size()]
                a = tmp_pool.tile(shp, F32, tag=tmp_tag)
                if bias_ap is None:
                    nc.scalar.activation(a[:], src, ACTF.Abs)
                    nc.vector.tensor_scalar(dst, src, 0.0, scalar2=None, op0=ALU.max)
                else:
                    nc.scalar.activation(a[:], src, ACTF.Abs, bias=bias_ap)
                    nc.vector.tensor_scalar(dst, src, bias_ap, scalar2=0.0,
                                            op0=ALU.add, op1=ALU.max)
                nc.scalar.activation(a[:], a[:], ACTF.Exp, scale=-1.0)
                nc.scalar.activation(a[:], a[:], ACTF.Ln, bias=onec[:, 0:1])
                nc.vector.tensor_tensor(dst, dst, a[:], op=ALU.add)

            stp_raw = smlp.tile([128, T * NT], F32, tag="stpraw")
            nc.sync.dma_start(out=stp_raw[:], in_=stp_d[:])
            softplus(stp_sb, stp_raw[:], None, smlp, "sptmp")
            nc.vector.tensor_scalar_mul(stpn_sb, stp_sb, -1.0)

            # ---------- dense projections ----------
            # cpT [16, K] = Ws1c.T @ context^T + bs1, replicated 8x on partitions
            cpT = cpTrep[0:EH, :]
            for nt4 in range(4):
                nsl = slice(nt4 * 512, (nt4 + 1) * 512)
                ps = psA.tile([128, 512], F32, space="PSUM", tag="mmps")
                nc.tensor.matmul(ps[:EH, :], bs1_sb[:1, :], ones1[:1, :512],
                                 start=True, stop=False)
                for dc in range(4):
                    lh = ldp.tile([128, EH], F32, tag="lhst16", bufs=1)
                    nc.sync.dma_start(out=lh[:],
                                      in_=Ws1c_d[dc * 128:(dc + 1) * 128, :])
                    rh = ldp.tile([128, 512], F32, tag="ctchunk", bufs=2)
                    nc.sync.dma_start(out=rh[:], in_=cT[dc * 128:(dc + 1) * 128, nsl])
                    nc.tensor.matmul(ps[:EH, :], lh[:], rh[:],
                                     start=False, stop=(dc == 3))
                nc.vector.tensor_copy(cpT[:, nsl], ps[:EH, :])
            for ls in range(1, 8):
                nc.sync.dma_start(out=cpTrep[ls * 16:(ls + 1) * 16, :], in_=cpT[:, :])

            # trio projection first (scores need only trio+cpT), then u
            for lt in range(NT):
                pst3 = psC.tile([128, 512], F32, space="PSUM", tag="ctx2")
                for dc in range(4):
                    lh = ldp.tile([128, 128], F32, tag="lhst", bufs=3)
                    nc.sync.dma_start(
                        out=lh[:], in_=tT[dc * 128:(dc + 1) * 128,
                                          lt * 128:(lt + 1) * 128])
                    nc.tensor.matmul(pst3[:, :48], lh[:],
                                     Wtr3_sb[:, dc * 48:(dc + 1) * 48],
                                     start=(dc == 0), stop=(dc == 3))
                nc.scalar.copy(trio[lt][:], pst3[:, :48])
                # tpbT: per-octet score bias columns, partition p = ls*16 + e
                for oc in range(16):
                    nc.sync.dma_start(
                        out=tpbT[:, lt * 16 + oc:lt * 16 + oc + 1],
                        in_=trio[lt][oc * 8:(oc + 1) * 8, 0:EH])
            Wt_t = load_w(Wt_d)
            for lt in range(NT):
                psu = psA.tile([128, 512], F32, space="PSUM", tag="mmps")
                nc.tensor.matmul(psu[:, :], ones1[0:1, :128], bt_b[:1, :],
                                 start=True, stop=False)
                for dc in range(4):
                    lh = ldp.tile([128, 128], F32, tag="lhst", bufs=3)
                    nc.sync.dma_start(
                        out=lh[:], in_=tT[dc * 128:(dc + 1) * 128,
                                          lt * 128:(lt + 1) * 128])
                    nc.tensor.matmul(psu[:, :], lh[:],
                                     Wt_t[:, dc * D:(dc + 1) * D],
                                     start=False, stop=(dc == 3))
                nc.scalar.copy(u_sb[lt][:], psu[:, :])

            # context projection -> Tctx, bf16 single-pass matmuls
            Wcb_t = wp.tile([128, 4 * D], BF16, tag="wrhsb")
            for dc in range(4):
                nc.sync.dma_start(out=Wcb_t[:, dc * D:(dc + 1) * D],
                                  in_=Wcb_d[dc * 128:(dc + 1) * 128, :])

            def emit_ctx_kt(kt):
                psv = psA.tile([128, 512], F32, space="PSUM", tag="mmps")
                psa = psC.tile([128, 512], F32, space="PSUM", tag="ctx2")
                nc.tensor.matmul(psv[:, :], ones1[32:33, :128], bc_b[:1, :],
                                 start=True, stop=False)
                nc.tensor.matmul(psa[:, :32], ones1[0:1, :128], bacl_sb[:1, :],
                                 start=True, stop=False)
                for dc in range(4):
                    lh = ldp.tile([128, 128], BF16, tag="lhstb")
                    nc.sync.dma_start(
                        out=lh[:], in_=cTb_d[dc * 128:(dc + 1) * 128,
                                            kt * 128:(kt + 1) * 128])
                    nc.tensor.matmul(psv[:, :], lh[:],
                                     Wcb_t[:, dc * D:(dc + 1) * D],
                                     start=False, stop=(dc == 3))
                    nc.tensor.matmul(psa[:, :32], lh[:],
                                     Waclb_sb[:, dc * 32:(dc + 1) * 32],
                                     start=False, stop=(dc == 3))
                stg = ldp.tile([128, CROW], BF16, tag="stgb")
                nc.scalar.copy(stg[:, 0:D], psv[:, :])
                nc.scalar.copy(stg[:, D:D + 32], psa[:, :32])
                nc.sync.dma_start(out=Tctx[kt * 128:(kt + 1) * 128, :],
                                  in_=stg[:, :])

            # ---------- score phase ----------
            class ScoreEmitter:
                """Stepwise score emission so octets can interleave into the
                t-loop of the previous l-tile (fills PE/ACT while DVE works)."""

                def __init__(self, lt, filler=None):
                    self.lt = lt
                    self.filler = filler
                    self.oc = 0
                    self.scores = medp.tile([128, K], F32, tag="scores",
                                            bufs=1)

                def step(self, n=1):
                    lt = self.lt
                    for oc in range(self.oc, min(self.oc + n, 16)):
                        if self.filler is not None:
                            self.filler(oc)
                        for hf in range(2):
                            g_sc = medp.tile([128, K // 2], F32, tag="gsc",
                                             bufs=2)
                            nc.scalar.activation(
                                g_sc[:], cpTrep[:, hf * 1024:(hf + 1) * 1024],
                                ACTF.Gelu,
                                bias=tpbT[:, lt * 16 + oc:lt * 16 + oc + 1])
                            for nq in range(2):
                                col = hf * 1024 + nq * 512
                                pssc = psS.tile([8, 512], F32, space="PSUM",
                                                tag="small")
                                nc.tensor.matmul(pssc[:, :], Ws2bd_sb[:],
                                                 g_sc[:, nq * 512:(nq + 1) * 512],
                                                 start=True, stop=True)
                                sstg = medp.tile([8, 512], F32, tag="sstg",
                                                 bufs=2)
                                if oc % 2 == 0:
                                    nc.scalar.copy(sstg[:], pssc[:, :])
                                else:
                                    nc.vector.tensor_copy(sstg[:], pssc[:, :])
                                nc.sync.dma_start(
                                    out=self.scores[oc * 8:(oc + 1) * 8,
                                                    col:col + 512],
                                    in_=sstg[:])
                    self.oc = min(self.oc + n, 16)

                def finish(self):
                    self.step(16 - self.oc)
                    mx8 = smlp.tile([128, 8], F32, tag="mx8")
                    idx = smlp.tile([128, 8], U32, tag="idx", bufs=2)
                    nc.vector.max(out=mx8[:], in_=self.scores[:])
                    nc.vector.max_index(out=idx[:], in_max=mx8[:],
                                        in_values=self.scores[:])
                    return idx

            def emit_gather(idx):
                gb = gbp.tile([128, WWIN * CROW], BF16, tag="gb")
                for w in range(WWIN):
                    nc.gpsimd.indirect_dma_start(
                        out=gb[:, w * CROW:(w + 1) * CROW],
                        out_offset=None,
                        in_=Tctx[:, :],
                        in_offset=bass.IndirectOffsetOnAxis(ap=idx[:, w:w + 1],
                                                            axis=0),
                    )
                return gb

            # score-0 interleaves the Tctx context projection as filler
            em0 = ScoreEmitter(0, filler=emit_ctx_kt)
            idx_next = em0.finish()
            gb_next = emit_gather(idx_next)

            # ---------- per l-tile ----------
            for lt in range(NT):
                idx = idx_next
                gb = gb_next
                gbv = gb[:].rearrange("p (w c) -> p w c", w=8)
                # next l-tile's scores drip-fed into this tile's t-loop
                em = ScoreEmitter(lt + 1) if lt + 1 < NT else None

                # ----- per-edge angles -----
                jf = smlp.tile([128, 8], F32, tag="jf")
                nc.vector.tensor_copy(jf[:], idx[:])
                delta = smlp.tile([128, 8], F32, tag="delta")
                nc.vector.tensor_scalar(delta[:], jf[:], lcol_sb[:, lt:lt + 1],
                                        scalar2=None, op0=ALU.subtract)
                ang = medp.tile([128, 8 * 32], F32, tag="ang", bufs=1)
                nc.vector.tensor_tensor(
                    out=ang[:].rearrange("p (w f) -> p w f", w=8),
                    in0=delta[:].unsqueeze(2).to_broadcast((128, 8, 32)),
                    in1=invf_sb[:].unsqueeze(1).to_broadcast((128, 8, 32)),
                    op=ALU.mult)
                # range-reduce to [-pi, pi]
                MAGIC = 1.5 * 2.0 ** 23
                angt = medp.tile([128, 8 * 32], F32, tag="angt", bufs=1)
                nc.vector.tensor_scalar_mul(angt[:], ang[:], 1.0 / TWO_PI)
                angr = medp.tile([128, 8 * 32], F32, tag="angr", bufs=1)
                nc.vector.tensor_scalar(angr[:], angt[:], MAGIC, scalar2=MAGIC,
                                        op0=ALU.add, op1=ALU.subtract)
                nc.vector.tensor_sub(angt[:], angt[:], angr[:])
                nc.vector.tensor_scalar_mul(ang[:], angt[:], TWO_PI)
                # sinb = sin(-ang) = sin(ref-ang); cosb = cos via sin(pi/2-|ang|)
                cosb = medp.tile([128, 8 * 32], BF16, tag="cosb")
                sinb = medp.tile([128, 8 * 32], BF16, tag="sinb")
                nc.scalar.activation(sinb[:], ang[:], ACTF.Sin, scale=-1.0)
                nc.vector.tensor_scalar_mul(angr[:], ang[:], -1.0)
                nc.vector.tensor_max(angt[:], ang[:], angr[:])
                nc.scalar.activation(cosb[:], angt[:], ACTF.Sin, scale=-1.0,
                                     bias=halfpi[:, 0:1])

                # ----- alphas = softplus(gelu(ta+ca) @ Wa2 + ba2) -----
                ha = smlp.tile([128, 8 * EH], F32, tag="ha")
                nc.vector.tensor_tensor(
                    out=ha[:].rearrange("p (w c) -> p w c", w=8),
                    in0=trio[lt][:, 16:32].unsqueeze(1).to_broadcast((128, 8, EH)),
                    in1=gbv[:, :, D:D + EH],
                    op=ALU.add)
                nc.scalar.activation(ha[:], ha[:], ACTF.Gelu)
                haw = smlp.tile([128, 8 * EH], F32, tag="haw")
                nc.vector.tensor_tensor(
                    out=haw[:].rearrange("p (w c) -> p w c", w=8),
                    in0=ha[:].rearrange("p (w c) -> p w c", w=8),
                    in1=wa2_sb[:].unsqueeze(1).to_broadcast((128, 8, EH)),
                    op=ALU.mult)
                alphas = smlp.tile([128, 8], F32, tag="alphas")
                nc.vector.tensor_reduce(alphas[:], haw[:].rearrange(
                    "p (w c) -> p w c", w=8), axis=AX.X, op=ALU.add)
                softplus(alphas[:], alphas[:], ba2_sb[:, 0:1], smlp, "sptmp")
                alphab = smlp.tile([128, 8], BF16, tag="alphab")
                nc.scalar.copy(alphab[:], alphas[:])

                # ----- g17 = [gelu(tl + cl) | 1], per-w transposes -----
                gmat17 = smlp.tile([128, 8 * KH], F32, tag="gmat17", bufs=1)
                g3 = gmat17[:].rearrange("p (w c) -> p w c", w=8)
                # gelu(tl + cl): add then gelu
                nc.vector.tensor_tensor(
                    out=g3[:, :, 0:EH],
                    in0=trio[lt][:, 32:48].unsqueeze(1).to_broadcast((128, 8, EH)),
                    in1=gbv[:, :, D + EH:D + 2 * EH],
                    op=ALU.add)
                nc.scalar.activation(g3[:, :, 0:EH], g3[:, :, 0:EH], ACTF.Gelu)
                nc.vector.memset(g3[:, :, EH:KH], 1.0)

                # gT4 [4 sectors x 17 rows, 2 quads x 128]: g17^T per w
                gT4 = gbp.tile([128, 2 * 128], BF16, tag="gT4")
                nc.vector.memset(gT4[:], 0.0)
                for w in range(WWIN):
                    q, s = w // 4, w % 4
                    pst = psS.tile([KH, 128], F32, space="PSUM", tag="small")
                    nc.tensor.transpose(
                        out=pst[:, :], in_=g3[:, w, :], identity=ident)
                    nc.vector.tensor_copy(
                        gT4[32 * s:32 * s + KH, q * 128:(q + 1) * 128], pst[:, :])

                # grep [p, (w r k)] bf16; gkw [p, (k w)] bf16
                grep = smlp.tile([128, WWIN * KR], BF16, tag="grep", bufs=2)
                nc.vector.tensor_copy(
                    grep[:].rearrange("p (w r k) -> p w r k", w=8, r=R),
                    g3.unsqueeze(2).to_broadcast((128, 8, R, KH)))
                gkw = smlp.tile([128, KH * WWIN], BF16, tag="gkw", bufs=2)
                nc.vector.tensor_copy(
                    gkw[:].rearrange("p (k w) -> p k w", k=KH),
                    gmat17[:].rearrange("p (w k) -> p k w", w=8))

                # ----- norms: GG = g17 @ G2 (PE), nrm2 = g17 . GG (DVE) -----
                GGsb = bigp.tile([128, WWIN * H * KR], BF16, tag="big")
                for w in range(WWIN):
                    q, s = w // 4, w % 4
                    for ch, (c0, cw) in enumerate(((0, 512), (512, 512), (1024, 64))):
                        psg = psA.tile([128, 512], F32, space="PSUM", tag="mmps")
                        nc.tensor.matmul(
                            psg[:, :cw],
                            gT4[32 * s:32 * s + 32, q * 128:(q + 1) * 128],
                            G2_sb[32 * s:32 * s + 32, c0:c0 + cw],
                            start=True, stop=True, tile_position=(32 * s, 0))
                        nc.scalar.copy(
                            GGsb[:, w * H * KR + c0:w * H * KR + c0 + cw],
                            psg[:, :cw])
                nrm2 = smlp.tile([128, WWIN * H * R], F32, tag="nrm2", bufs=1)
                n3 = nrm2[:].rearrange("p (w h r) -> p w h r", w=8, h=H)
                for w in range(WWIN):
                    gg_w = GGsb[:].rearrange(
                        "p (w hr k) -> p w hr k", w=8, k=KH)[:, w]
                    nc.vector.tensor_tensor(
                        out=gg_w,
                        in0=gg_w,
                        in1=grep[:].rearrange(
                            "p (w r k) -> p w r k", w=8, r=R)[:, w, 0:1, :]
                            .to_broadcast((128, H * R, KH)),
                        op=ALU.mult)
                    nc.vector.tensor_reduce(
                        n3[:, w], gg_w, axis=AX.X, op=ALU.add)
                rec = smlp.tile([128, WWIN * H * R], F32, tag="rec", bufs=2)
                nc.vector.tensor_scalar_max(rec[:], nrm2[:], 1e-24)
                nc.vector.reciprocal(rec[:], rec[:])
                # rec viewed (h, r, w) for the y-scale
                recv = rec[:].rearrange("p (w h r) -> p h r w", w=8, h=H)

                # ----- t-loop -----
                for t in range(T):
                    stc = slice(t * NT + lt, t * NT + lt + 1)
                    if t == T - 1 and em is not None:
                        # next tile's topk + gather hide under this t-pass
                        idx_next = em.finish()
                        gb_next = emit_gather(idx_next)
                    # u casts: ub = bf16(u); ubrot = rot_half(u) = [-u_hi, u_lo]
                    ub = medp.tile([128, D], BF16, tag="ub", bufs=2)
                    nc.scalar.copy(ub[:], u_sb[lt][:])
                    ubrot = medp.tile([128, D], BF16, tag="ubrot", bufs=2)
                    ur3 = ubrot[:].rearrange("p (h a b) -> p h a b", h=H, a=2)
                    uv3 = u_sb[lt][:].rearrange("p (h a b) -> p h a b", h=H, a=2)
                    nc.scalar.activation(ur3[:, :, 0, :], uv3[:, :, 1, :],
                                         ACTF.Copy, scale=-1.0)
                    nc.scalar.copy(ur3[:, :, 1, :], uv3[:, :, 0, :])

                    # diff [p, (w h d)] bf16
                    t0 = medp.tile([128, WWIN * D], BF16, tag="t0", bufs=1)
                    nc.vector.tensor_tensor(
                        out=t0[:].rearrange("p (w g b) -> p w g b", w=8, g=2 * H),
                        in0=ub[:].rearrange("p (g b) -> p g b", g=2 * H)
                            .unsqueeze(1).to_broadcast((128, 8, 2 * H, 32)),
                        in1=cosb[:].rearrange("p (w f) -> p w f", w=8)
                            .unsqueeze(2).to_broadcast((128, 8, 2 * H, 32)),
                        op=ALU.mult)
                    t1 = medp.tile([128, WWIN * D], BF16, tag="t1", bufs=1)
                    nc.vector.tensor_tensor(
                        out=t1[:].rearrange("p (w g b) -> p w g b", w=8, g=2 * H),
                        in0=ubrot[:].rearrange("p (g b) -> p g b", g=2 * H)
                            .unsqueeze(1).to_broadcast((128, 8, 2 * H, 32)),
                        in1=sinb[:].rearrange("p (w f) -> p w f", w=8)
                            .unsqueeze(2).to_broadcast((128, 8, 2 * H, 32)),
                        op=ALU.mult)
                    nc.vector.tensor_tensor(out=t0[:], in0=t0[:], in1=t1[:],
                                            op=ALU.add)
                    diff = medp.tile([128, WWIN * D], BF16, tag="diff", bufs=1)
                    nc.vector.tensor_tensor(
                        out=diff[:].rearrange("p (w e) -> p w e", w=8),
                        in0=t0[:].rearrange("p (w e) -> p w e", w=8),
                        in1=gbv[:, :, 0:D],
                        op=ALU.subtract)

                    # alpha term early (DVE busy while PE transposes)
                    adiff = medp.tile([128, WWIN * D], BF16, tag="adiff", bufs=1)
                    nc.vector.tensor_tensor(
                        out=adiff[:].rearrange("p (w e) -> p w e", w=8),
                        in0=diff[:].rearrange("p (w e) -> p w e", w=8),
                        in1=alphab[:].unsqueeze(2).to_broadcast((128, 8, D)),
                        op=ALU.mult)
                    rsa = smlp.tile([128, D], F32, tag="rsa", bufs=1)
                    nc.vector.tensor_reduce(
                        rsa[:],
                        adiff[:].rearrange("p (w e) -> p e w", w=8),
                        axis=AX.X, op=ALU.add)

                    # diff^T per (w, head-pair): [128,(128)] -> [128 rows of (h2 d)]
                    dT = medp.tile([128, WWIN * D], BF16, tag="dT", bufs=1)
                    d4 = dT[:].rearrange("p (w hp e) -> p w hp e", w=8, hp=4)
                    dv4 = diff[:].rearrange("p (w hp e) -> p w hp e", w=8, hp=4)
                    for w in range(WWIN):
                        for hp in range(4):
                            pst = psT.tile([128, 128], BF16, space="PSUM",
                                           tag="tq")
                            nc.tensor.transpose(
                                out=pst[:, :], in_=dv4[:, w, hp, :],
                                identity=identb)
                            nc.scalar.copy(d4[:, w, hp, :], pst[:, :])

                    if em is not None and t == 0:
                        em.step(4)

                    # Q pair matmuls: psQ[row, 2*KR] per (w, hp)
                    Qsb = bigp.tile([128, H * WWIN * KR], BF16, tag="big")
                    q4 = Qsb[:].rearrange("p (h w rk) -> p h w rk", h=H, w=8)
                    for w in range(WWIN):
                        for hp in range(4):
                            psq = psT.tile([128, 2 * KR], F32, space="PSUM",
                                           tag="tq")
                            nc.tensor.matmul(
                                psq[:, :], d4[:, w, hp, :],
                                W2Tp_sb[:, hp * 2 * KR:(hp + 1) * 2 * KR],
                                start=True, stop=True)
                            # evac both heads of the pair
                            nc.scalar.copy(q4[:, 2 * hp, w, :], psq[:, 0:KR])
                            nc.scalar.copy(q4[:, 2 * hp + 1, w, :],
                                           psq[:, KR:2 * KR])

                    # filler: next tile's score octets run on PE/ACT while
                    # the einsum DVE phase below executes
                    if em is not None and t == 0:
                        em.step(4)

                    # einsum1: y[h,r,w] = sum_k g17 * Q  (normalized by rec)
                    # in-place product into Qsb, then segmented reduce over k
                    nc.vector.tensor_tensor(
                        out=Qsb[:].rearrange("p (h wrk) -> p h wrk", h=H),
                        in0=Qsb[:].rearrange("p (h wrk) -> p h wrk", h=H),
                        in1=grep[:].unsqueeze(1).to_broadcast(
                            (128, H, WWIN * KR)),
                        op=ALU.mult)
                    yt = smlp.tile([128, H * R * WWIN], F32, tag="yt", bufs=1)
                    yv = yt[:].rearrange("p (h r w) -> p h w r", h=H, r=R)
                    nc.vector.tensor_reduce(
                        yv,
                        Qsb[:].rearrange("p (hwr k) -> p hwr k", k=KH),
                        axis=AX.X, op=ALU.add)
                    ycast = smlp.tile([128, H * R * WWIN], BF16, tag="ycast", bufs=1)
                    nc.vector.tensor_tensor(
                        out=ycast[:].rearrange("p (h r w) -> p h r w",
                                               h=H, r=R),
                        in0=yt[:].rearrange("p (h r w) -> p h r w", h=H, r=R),
                        in1=recv,
                        op=ALU.mult)

                    # C[h, (k r)] = sum_w g17[w,k] y[h,w,r]
                    C = smlp.tile([128, H * KR], F32, tag="C", bufs=1)
                    c3 = C[:].rearrange("p (h k r) -> p h k r", h=H, k=KH)
                    yc3 = ycast[:].rearrange("p (h r w) -> p h r w", h=H, r=R)
                    gkw3 = gkw[:].rearrange("p (k w) -> p k w", k=KH)
                    for h in range(H):
                        prodC = medp.tile([128, KH * R * WWIN], BF16,
                                          tag="prodC", bufs=2)
                        nc.vector.tensor_tensor(
                            out=prodC[:].rearrange("p (k r w) -> p k r w",
                                                   k=KH, r=R),
                            in0=yc3[:, h].unsqueeze(1).to_broadcast(
                                (128, KH, R, WWIN)),
                            in1=gkw3.unsqueeze(2).to_broadcast(
                                (128, KH, R, WWIN)),
                            op=ALU.mult)
                        nc.vector.tensor_reduce(
                            c3[:, h],
                            prodC[:].rearrange("p (kr w) -> p kr w", w=8),
                            axis=AX.X, op=ALU.add)

                    # filler: more score octets ahead of the CT/res PE phase
                    if em is not None and t == 0:
                        em.step(4)

                    # C^T via PE; res = C^T-mm-W2C + bias part
                    psR = psRp.tile([128, D], F32, space="PSUM", tag="rps")
                    CTm = medp.tile([128, H * 128], BF16, tag="CTm", bufs=1)
                    CTb = medp.tile([8, H * 128], BF16, tag="CTb", bufs=1)
                    for h in range(H):
                        pct = psT.tile([128, 128], F32, space="PSUM", tag="tq")
                        nc.tensor.transpose(
                            out=pct[:, :], in_=C[:, h * KR:h * KR + 128],
                            identity=ident)
                        nc.scalar.copy(CTm[:, h * 128:(h + 1) * 128], pct[:, :])
                        pcb = psT.tile([8, 128], F32, space="PSUM", tag="tq")
                        nc.tensor.transpose(
                            out=pcb[:, :], in_=C[:, h * KR + 128:(h + 1) * KR],
                            identity=ident)
                        nc.scalar.copy(CTb[:, h * 128:(h + 1) * 128], pcb[:, :])
                    for h in range(H):
                        nc.tensor.matmul(
                            psR[:, h * HD:(h + 1) * HD],
                            CTm[:, h * 128:(h + 1) * 128],
                            W2C_sb[:, h * HD:(h + 1) * HD],
                            start=True, stop=False)
                        nc.tensor.matmul(
                            psR[:, h * HD:(h + 1) * HD],
                            CTb[0:8, h * 128:(h + 1) * 128],
                            W2C2_sb[0:8, h * HD:(h + 1) * HD],
                            start=False, stop=True)

                    # u -= step * (res + rsa)
                    tmpu = smlp.tile([128, D], F32, tag="tmpu", bufs=1)
                    nc.vector.tensor_tensor(tmpu[:], psR[:, :], rsa[:],
                                            op=ALU.add)
                    nc.vector.scalar_tensor_tensor(
                        out=u_sb[lt][:], in0=tmpu[:], scalar=stpn_sb[:, stc],
                        in1=u_sb[lt][:], op0=ALU.mult, op1=ALU.add)


            # ---------- output projection: y = u @ Wo + bo ----------
            Wo_t = load_w(Wo_d)
            for lt in range(NT):
                psy = psA.tile([128, 512], F32, space="PSUM", tag="mmps")
                nc.tensor.matmul(psy[:, :], ones1[64:65, :128], bo_b[:1, :],
                                 start=True, stop=False)
                for dc in range(4):
                    pst = psS.tile([128, 128], F32, space="PSUM", tag="small")
                    nc.tensor.transpose(
                        out=pst[:, :], in_=u_sb[lt][:, dc * 128:(dc + 1) * 128],
                        identity=ident)
                    uT = ldp.tile([128, 128], F32, tag="uT", bufs=1)
                    nc.scalar.copy(uT[:], pst[:, :])
                    nc.tensor.matmul(psy[:, :], uT[:], Wo_t[:, dc * D:(dc + 1) * D],
                                     start=False, stop=(dc == 3))
                ystg = ldp.tile([128, 512], F32, tag="stg", bufs=1)
                nc.scalar.copy(ystg[:], psy[:, :])
                nc.sync.dma_start(out=y_d[lt * 128:(lt + 1) * 128, :], in_=ystg[:])

    nc.finalize()
    return nc


def make_in_maps(inputs):
    """Host-side prep: slice/transpose inputs into the 8 per-core input maps."""
    target = np.asarray(inputs["target"], np.float32)
    context = np.asarray(inputs["context"], np.float32)
    Wt = np.asarray(inputs["Wt"], np.float32)
    bt = np.asarray(inputs["bt"], np.float32)
    Wc = np.asarray(inputs["Wc"], np.float32)
    bc = np.asarray(inputs["bc"], np.float32)
    Ws1 = np.asarray(inputs["Ws1"], np.float32)
    bs1 = np.asarray(inputs["bs1"], np.float32)
    Ws2 = np.asarray(inputs["Ws2"], np.float32)
    Wa1 = np.asarray(inputs["Wa1"], np.float32)
    ba1 = np.asarray(inputs["ba1"], np.float32)
    Wa2 = np.asarray(inputs["Wa2"], np.float32)
    ba2 = np.asarray(inputs["ba2"], np.float32)
    Wl1 = np.asarray(inputs["Wl1"], np.float32)
    bl1 = np.asarray(inputs["bl1"], np.float32)
    Wl2 = np.asarray(inputs["Wl2"], np.float32)
    bl2 = np.asarray(inputs["bl2"], np.float32)
    step_sizes = np.asarray(inputs["step_sizes"], np.float32)
    Wo = np.asarray(inputs["Wo"], np.float32)
    bo = np.asarray(inputs["bo"], np.float32)

    import ml_dtypes
    Ws2bd = np.zeros((128, 8), np.float32)
    for ls in range(8):
        Ws2bd[ls * 16:(ls + 1) * 16, ls] = Ws2[:, 0]

    # v4 factored tables
    W2full = np.zeros((KH, H, R, HD), np.float32)
    W2full[:16] = Wl2.reshape(EH, H, R, HD)
    W2full[16] = bl2.reshape(H, R, HD)
    # W2Tp [128, 4*2*KR]: rows (a,d); cols (hp, b, r, k); block-diag in (a==b)
    W2Tp = np.zeros((128, 4 * 2 * KR), np.float32)
    for hp in range(4):
        for bb in range(2):
            h = 2 * hp + bb
            # [d, r, k] table for head h
            tbl = W2full[:, h].transpose(2, 1, 0).reshape(HD, KR)  # d,(r,k)
            W2Tp[bb * HD:(bb + 1) * HD,
                 hp * 2 * KR + bb * KR:hp * 2 * KR + (bb + 1) * KR] = tbl
    # W2C [128, H*HD]: rows (k<16)*8+r; cols h*64+d
    W2C = np.zeros((128, H * HD), np.float32)
    for k in range(EH):
        for r in range(R):
            W2C[k * 8 + r] = W2full[k, :, r, :].reshape(H * HD)
    # W2C2 [8, H*HD]: rows r; cols h*64+d (k=16 bias part)
    W2C2 = np.zeros((8, H * HD), np.float32)
    for r in range(R):
        W2C2[r] = W2full[16, :, r, :].reshape(H * HD)
    # G2 [128, H*KR]: rows (sector s)*32 + k'; cols h*KR + r*KH + k
    G2f = np.einsum("ahrd,bhrd->abhr", W2full, W2full)  # [17,17,H,R]
    G2s = np.zeros((32, H * KR), np.float32)
    G2s[:KH] = G2f.transpose(0, 2, 3, 1).reshape(KH, H * R * KH)
    G2 = np.tile(G2s, (4, 1))

    invf = (1.0 / (10000.0 ** (np.arange(0, HD, 2, dtype=np.float32) / HD)))[None, :]
    bpack = np.zeros((128, D), np.float32)
    bpack[0] = bt
    bpack[32] = bc
    bpack[64] = bo

    common = dict(
        Wt=Wt, Wcb=Wc.astype(ml_dtypes.bfloat16), Wo=Wo, bpack=bpack,
        Wtr3=np.ascontiguousarray(np.concatenate([Ws1[:D], Wa1[:D], Wl1[:D]], axis=1)),
        Ws1c=np.ascontiguousarray(Ws1[D:]),
        Waclb=np.ascontiguousarray(
            np.concatenate([Wa1[D:], Wl1[D:]], axis=1)).astype(ml_dtypes.bfloat16),
        bs1=bs1[None, :],
        bacl=np.concatenate([ba1, bl1])[None, :],
        Ws2bd=Ws2bd, Wa2=np.ascontiguousarray(Wa2.T),
        ba2=np.asarray(ba2, np.float32).reshape(1, 1),
        W2Tp=W2Tp.astype(ml_dtypes.bfloat16),
        W2C=W2C.astype(ml_dtypes.bfloat16),
        W2C2=W2C2.astype(ml_dtypes.bfloat16),
        G2=G2.astype(ml_dtypes.bfloat16),
        invf=np.ascontiguousarray(invf, np.float32),
    )

    in_maps = []
    for c in range(8):
        b, rc = c // 4, c % 4
        rows = slice(rc * LC, (rc + 1) * LC)
        stp = np.ascontiguousarray(
            step_sizes[:, rows].reshape(T, NT, 128).transpose(2, 0, 1)
            .reshape(128, T * NT))
        lcol = np.ascontiguousarray(
            (rc * LC + np.arange(LC, dtype=np.float32)).reshape(NT, 128).T)
        m = dict(common)
        cTf = np.ascontiguousarray(context[b].T)
        m.update(
            tT=np.ascontiguousarray(target[b, rows].T),
            cT=cTf, cTb=cTf.astype(ml_dtypes.bfloat16),
            stp=stp, lcol=lcol,
        )
        in_maps.append(m)
    return in_maps


_NC_CACHE = {}


def kernel(**inputs):
    if "nc" not in _NC_CACHE:
        _NC_CACHE["nc"] = build_program()
    nc = _NC_CACHE["nc"]
    in_maps = make_in_maps(inputs)
    res = run_bass_kernel_spmd(nc, in_maps, list(range(8)))
    out = np.empty((B, L, D), np.float32)
    for c in range(8):
        b, rc = c // 4, c % 4
        out[b, rc * LC:(rc + 1) * LC] = res.results[c]["y"]
    return out
